# revision 29
# baseline (speedup 1.0000x reference)
"""Trainium2 Bass kernel for nn_EqLayerNodeAttr (gnn message passing).

Strategy:
  - Edges sharded across 8 cores by whole destination-node (col) groups, so
    each core owns a disjoint set of output rows -> no collectives.
  - Within a core, edges are packed into tiles of <=512 edges covering <=64
    distinct destination nodes.  Per tile:
      * src node rows gathered with one multi-offset indirect DMA (bf16 table)
      * dst node rows: the <=64 distinct rows are gathered once ("window"),
        then expanded per-edge with a one-hot matmul on the PE
      * per-edge 2x2 rotations on DVE with broadcast access patterns
      * features transposed to [feat, edge] layout via PE transposes
      * 608->256->192 MLP as bf16 matmuls with fp32 PSUM accumulation
      * messages rotated back per edge, then segment-summed over the tile's
        <=64 destinations with a one-hot matmul and written to the output
        rows with an indirect scatter DMA (each dst row written exactly once
        globally -> no read-modify-write races).
"""

import numpy as np
import ml_dtypes

# ---- problem constants (hardcoded per contract) ----
N = 10000
E = 160000
L = 4
NS, NSA = 64, 16
NR, NRA = 16, 8
DIST = 64
HID = 256
SCAL = NS + NSA            # 80
NREP = NR + NRA            # 24
ROTF = NREP * 2 * L        # 192
FEAT = SCAL + ROTF         # 272
ROTD = ROTF * 2            # 384, l-duplicated rot features (j,k,m,l)
FEATD = SCAL + ROTD        # 464, node table row with dup rot part
DIN = 2 * FEAT + DIST      # 608
DOUT = NS + NR * 2 * L     # 192
DOUTD = NS + NR * L * 4    # 320, MLP2 out with dup rot part (j,k,m,l)
NCORES = 8

TP = 512                   # edges per tile
SUB = 128                  # edges per sub-tile
NSUBT = TP // SUB          # 4
W = 64                     # max distinct destination nodes per tile
NACC = N + W               # junk rows N..N+W-1 absorb padding writes
MW = 74                    # packed metadata words per lane
LROWS = 2048               # core-local output rows (owned cols + junk window)

BF16 = ml_dtypes.bfloat16

# K-chunks of the MLP input (W1 rows reordered to match, see _w1_chunks):
#  c0: dst_rot[0:128]            (featT block 0)
#  c1: dst_rot[128:192] p0:64  | src_rot[128:192] p64:128   (featT block 1)
#  c2: src_rot[0:128]            (featT block 2)
#  c3: dst_scal[0:80]            (sdst tile)
#  c4: src_scal[0:80]            (ssrc tile)
#  c5: dist[0:64] + ones row 64 carrying b1  (dist tile)
KC = [128, 128, 128, SCAL, SCAL, DIST + 1]

# single packed bf16 input tensor: rows of 512 cols
#  [0, NACC)                nodes (row padded 464 -> 512)
#  [R_W1, +6*128)           W1 chunks, cols 0:HID; chunk 5 row 64 = b1
#  [R_W2, +2*128)           W2 chunks, cols 0:DOUTD
#  [R_META(T), +T*128)      per-tile meta, raw-bitcast i32->bf16, cols 0:146
#  [R_DIST(T), +T*65)       per-tile distT (64 rows) + ones row, cols 0:512
PACK_W = 512
R_W1 = NACC
R_W2 = R_W1 + 6 * 128


def _pack_rows(T):
    r_meta = R_W2 + 2 * 128
    r_dist = r_meta + T * SUB
    rows = r_dist + T * (DIST + 1)
    return r_meta, r_dist, rows


def _w1_chunks():
    dst_scal = np.arange(0, 80)
    dst_rot = np.arange(80, 272)
    src_scal = np.arange(272, 352)
    src_rot = np.arange(352, 544)
    dist = np.arange(544, 608)
    return [
        dst_rot[0:128],
        np.concatenate([dst_rot[128:192], src_rot[128:192]]),
        src_rot[0:128],
        dst_scal,
        src_scal,
        dist,
    ]


# --------------------------------------------------------------------------
# host-side sharding / tiling
# --------------------------------------------------------------------------

def _shard_and_tile(row, col):
    """Group edges by destination col; split whole cols across 8 cores with
    balanced edge counts; pack each core's cols into (<=TP edges, <=W cols)
    tiles."""
    order = np.argsort(col, kind="stable")
    scol = col[order]
    uniq, starts = np.unique(scol, return_index=True)
    starts = np.append(starts, len(scol))

    per_core_tiles = [[] for _ in range(NCORES)]
    core_cols = [[] for _ in range(NCORES)]
    target = len(scol) / NCORES
    ci = 0
    for ui in range(len(uniq)):
        lo = starts[ui]
        while ci < NCORES - 1 and lo >= (ci + 1) * target:
            ci += 1
        core_cols[ci].append(ui)

    for c in range(NCORES):
        tiles = []
        cur_e, cur_c = [], []
        for ui in core_cols[c]:
            lo, hi = starts[ui], starts[ui + 1]
            deg = hi - lo
            if deg > TP:
                raise ValueError("col degree exceeds tile capacity")
            if cur_e and (len(cur_e) + deg > TP or len(cur_c) + 1 > W):
                tiles.append((np.array(cur_e, np.int64), np.array(cur_c, np.int64)))
                cur_e, cur_c = [], []
            cur_e.extend(order[lo:hi].tolist())
            cur_c.append(int(uniq[ui]))
        if cur_e:
            tiles.append((np.array(cur_e, np.int64), np.array(cur_c, np.int64)))
        per_core_tiles[c] = tiles
    return per_core_tiles


def _host_prep(inputs):
    x_scalar = np.asarray(inputs["x_scalar"], np.float32)
    x_rot = np.asarray(inputs["x_rot"], np.float32)
    na_scalar = np.asarray(inputs["na_scalar"], np.float32)
    na_rot = np.asarray(inputs["na_rot"], np.float32)
    edge_index = np.asarray(inputs["edge_index"])
    dist_emb = np.asarray(inputs["dist_emb"], np.float32)
    rot = np.asarray(inputs["rot"], np.float32)
    W1 = np.asarray(inputs["W1"], np.float32)
    b1 = np.asarray(inputs["b1"], np.float32)
    W2 = np.asarray(inputs["W2"], np.float32)
    b2 = np.asarray(inputs["b2"], np.float32)

    row = edge_index[0].astype(np.int64)
    col = edge_index[1].astype(np.int64)

    # node table rows: [scal 80 | xr dup over l, order (j,k,m,l), 384]
    xs = np.concatenate([x_scalar, na_scalar], axis=1)                  # [N, 80]
    xr3 = np.concatenate([x_rot, na_rot], axis=1).reshape(N, NREP, L, 2)
    xr_dup = np.repeat(xr3[..., None], 2, axis=-1).reshape(N, ROTD)
    nodes = np.zeros((NACC, FEATD), np.float32)
    nodes[:N] = np.concatenate([xs, xr_dup], axis=1)
    nodes_bf16 = nodes.astype(BF16)

    per_core_tiles = _shard_and_tile(row, col)
    T = max(len(t) for t in per_core_tiles)
    r_meta, r_dist, rows_total = _pack_rows(T)

    W1c = np.zeros((6, 128, HID), np.float32)
    for c, idx in enumerate(_w1_chunks()):
        W1c[c, : len(idx)] = W1[idx]
    W1c[5, DIST] = b1          # ones row in the dist chunk carries the bias
    W1c = W1c.astype(BF16)
    # W2 cols: [scal 64 | dup over l, order (j,k,m,l), 256]
    W2r = W2[:, NS:].reshape(HID, NR, L, 2)
    W2d = np.concatenate(
        [W2[:, :NS], np.repeat(W2r[..., None], 2, axis=-1).reshape(HID, 256)],
        axis=1,
    )
    W2c = W2d.reshape(2, 128, DOUTD).astype(BF16)

    # per-edge rote arrangements (bf16):
    #  fwd:  value rot[k,l,m] stored at (k,m,l)  -> transpose last two axes
    #  back: value rot[k,m,l] stored at (k,m,l)  -> natural order
    rot_fwd = np.ascontiguousarray(rot.transpose(0, 1, 3, 2)).reshape(-1, 16)
    rot_back = rot.reshape(-1, 16)

    per_core_inputs = []
    for c in range(NCORES):
        tiles = per_core_tiles[c]
        # packed per-lane metadata words:
        #  0:4 ridx | 4:8 crel | 8:40 rote_fwd (4 subs x 16 bf16)
        #  40:72 rote_back | 72 winrows (lanes 0..63)
        meta = np.zeros((T, SUB, MW), np.int32)
        meta[:, :, 4:8] = 127          # crel padding -> no onehot match
        dist = np.zeros((T, DIST + 1, TP), BF16)
        dist[:, DIST, :] = 1.0         # ones row multiplies the b1 row of W1
        rf_bf = np.zeros((T, SUB, NSUBT * 16), BF16)
        rb_bf = np.zeros((T, SUB, NSUBT * 16), BF16)
        # winrows: global col ids for the window NODE GATHER (junk -> zero
        # rows N..NACC).  winloc: core-local OUTPUT rows for the scatter,
        # owned cols get ids 0..n_owned-1 (ascending); junk at top of LROWS.
        owned = (
            np.concatenate([cols for _, cols in tiles])
            if tiles else np.zeros((0,), np.int64)
        )
        assert len(owned) <= LROWS - W, "owned cols exceed local output rows"
        winrows = np.tile(np.arange(W, dtype=np.int32) + N, (T, 1))
        winloc = np.tile(np.arange(W, dtype=np.int32) + (LROWS - W), (T, 1))
        loc0 = 0
        for t in range(T):
            if t >= len(tiles):
                continue
            eids, cols = tiles[t]
            ne, ncol = len(eids), len(cols)
            winrows[t, :ncol] = cols.astype(np.int32)
            winloc[t, :ncol] = loc0 + np.arange(ncol, dtype=np.int32)
            loc0 += ncol
            slot = np.arange(ne)
            lane, s = slot % SUB, slot // SUB
            m = meta[t]
            m[lane, s] = row[eids].astype(np.int32)
            m[lane, 4 + s] = np.searchsorted(cols, col[eids]).astype(np.int32)
            cidx = (s * 16)[:, None] + np.arange(16)
            rf_bf[t, lane[:, None], cidx] = rot_fwd[eids].astype(BF16)
            rb_bf[t, lane[:, None], cidx] = rot_back[eids].astype(BF16)
            dist[t, :DIST, :ne] = dist_emb[eids].T.astype(BF16)

        def pack(bf):
            u = bf.view(np.uint16).reshape(T, SUB, 32, 2).astype(np.uint32)
            return (u[..., 0] | (u[..., 1] << 16)).view(np.int32)

        meta[:, :, 8:40] = pack(rf_bf)
        meta[:, :, 40:72] = pack(rb_bf)
        meta[:, :W, 72] = winrows
        meta[:, :W, 73] = winloc

        allr = np.zeros((rows_total, PACK_W), BF16)
        allr[:NACC, :FEATD] = nodes_bf16
        allr[R_W1:R_W1 + 6 * 128, :HID] = W1c.reshape(6 * 128, HID)
        allr[R_W2:R_W2 + 2 * 128, :DOUTD] = W2c.reshape(2 * 128, DOUTD)
        meta_raw = meta.view(np.uint16).view(BF16).reshape(T * SUB, 2 * MW)
        allr[r_meta:r_meta + T * SUB, :2 * MW] = meta_raw
        allr[r_dist:r_dist + T * (DIST + 1), :] = dist.reshape(T * (DIST + 1), TP)
        per_core_inputs.append(dict(all=allr))

    meta_info = dict(per_core_tiles=per_core_tiles, row=row, col=col,
                     rot=rot, b2=b2)
    return per_core_inputs, T, meta_info


def _assemble(results, meta):
    col = meta["col"]
    deg = np.bincount(col, minlength=N)
    out = np.zeros((N, DOUT), np.float32)
    for c, tiles in enumerate(meta["per_core_tiles"]):
        acc = results[c]["acc"]
        if tiles:
            owned = np.concatenate([cols for _, cols in tiles])
            out[owned] = acc[: len(owned)]
    out[deg == 0] = 0.0
    b2 = meta["b2"]
    if np.any(b2):
        out[:, :NS] += np.outer(deg, b2[:NS])
        b2r = b2[NS:].reshape(NR, L, 2)
        rot = meta["rot"]
        corr = np.einsum("jkm,ekml->ejkl", b2r, rot).reshape(E, NR * 2 * L)
        np.add.at(out[:, NS:], col, corr)
    return out


# --------------------------------------------------------------------------
# device program
# --------------------------------------------------------------------------

def _build_program(T):
    from concourse import bacc, mybir
    import concourse.tile as tile
    from concourse.bass import IndirectOffsetOnAxis
    from concourse.masks import make_identity

    f32 = mybir.dt.float32
    bf16 = mybir.dt.bfloat16
    i32 = mybir.dt.int32
    AL = mybir.AluOpType
    ACTF = mybir.ActivationFunctionType

    nc = bacc.Bacc("TRN2", target_bir_lowering=False, debug=False)

    r_meta, r_dist, rows_total = _pack_rows(T)
    d_all = nc.dram_tensor("all", [rows_total, PACK_W], bf16, kind="ExternalInput").ap()
    d_acc = nc.dram_tensor("acc", [LROWS, DOUT], f32, kind="ExternalOutput").ap()
    # indirect_dma_start derives the index coefficient from the AP SHAPE
    # (prod of dims after the offset axis), so the gather AP must span the
    # full 512-wide rows for the coefficient to match the row stride.
    d_nodes = d_all[0:NACC, 0:PACK_W]

    with tile.TileContext(nc) as tc:
        with (
            tc.tile_pool(name="const", bufs=1) as cpool,
            tc.tile_pool(name="sb", bufs=3) as pool,
            tc.tile_pool(name="sb3", bufs=4) as pool3,
            tc.tile_pool(name="ph", bufs=2, space="PSUM") as pph,
            tc.tile_pool(name="po", bufs=1, space="PSUM") as ppo,
            tc.tile_pool(name="ptr", bufs=3, space="PSUM") as ptr,
            tc.tile_pool(name="px", bufs=1, space="PSUM") as px,
            tc.tile_pool(name="psc", bufs=1, space="PSUM") as psc,
        ):
            # ---- constants ----
            ident = cpool.tile([128, 128], bf16)
            make_identity(nc, ident[:])
            iota = cpool.tile([128, W], i32)
            nc.gpsimd.iota(iota[:], pattern=[[1, W]], base=0, channel_multiplier=0)
            w1sb = cpool.tile([128, 6 * HID], bf16)
            for c in range(6):
                nc.sync.dma_start(
                    out=w1sb[:, c * HID:(c + 1) * HID],
                    in_=d_all[R_W1 + c * 128:R_W1 + (c + 1) * 128, 0:HID],
                )
            w2sb = cpool.tile([128, 2 * DOUTD], bf16)
            for c in range(2):
                nc.sync.dma_start(
                    out=w2sb[:, c * DOUTD:(c + 1) * DOUTD],
                    in_=d_all[R_W2 + c * 128:R_W2 + (c + 1) * 128, 0:DOUTD],
                )

            def emit_front(t):
                # ---- per-tile loads ----
                meta_bf = pool.tile([SUB, 2 * MW], bf16)
                nc.sync.dma_start(
                    out=meta_bf[:],
                    in_=d_all[r_meta + t * SUB:r_meta + (t + 1) * SUB, 0:2 * MW],
                )
                dist_sb = pool.tile([DIST + 1, TP], bf16)
                nc.sync.dma_start(
                    out=dist_sb[:],
                    in_=d_all[r_dist + t * (DIST + 1):r_dist + (t + 1) * (DIST + 1), :],
                )
                ridx = meta_bf[:, 0:8].bitcast(i32)      # [128, 4]
                crel = meta_bf[:, 8:16].bitcast(i32)     # [128, 4]
                rote_f = meta_bf[:, 16:80]               # [128, 64]
                rote_b = meta_bf[:, 80:144]              # [128, 64]
                winr = meta_bf[0:W, 144:146].bitcast(i32)     # [W,1] global
                winr_sc = meta_bf[0:W, 146:148].bitcast(i32)  # [W,1] local

                # ---- one-hots (independent of gathers) ----
                onehot_e = pool.tile([SUB, NSUBT * W], bf16)
                onehot_w = pool.tile([W, TP], bf16)
                p_oh = ptr.tile([W, TP], bf16, tag="ptrans")
                for s in range(NSUBT):
                    oh_e = onehot_e[:, s * W:(s + 1) * W]
                    nc.vector.tensor_tensor(
                        out=oh_e,
                        in0=crel[:, s:s + 1].to_broadcast([SUB, W]),
                        in1=iota[:, :],
                        op=AL.is_equal,
                    )
                    nc.tensor.transpose(
                        out=p_oh[:, s * SUB:(s + 1) * SUB], in_=oh_e,
                        identity=ident[:],
                    )
                nc.scalar.activation(out=onehot_w[:], in_=p_oh[:], func=ACTF.Copy)

                # ---- gathers (full 512-wide rows; see d_nodes comment) ----
                win = pool.tile([W, PACK_W], bf16)
                nc.gpsimd.indirect_dma_start(
                    out=win[:],
                    out_offset=None,
                    in_=d_nodes,
                    in_offset=IndirectOffsetOnAxis(ap=winr, axis=0),
                )
                src_g = pool.tile([SUB, NSUBT * PACK_W], bf16)
                for s in range(NSUBT):
                    nc.gpsimd.indirect_dma_start(
                        out=src_g[:, s * PACK_W:(s + 1) * PACK_W],
                        out_offset=None,
                        in_=d_nodes,
                        in_offset=IndirectOffsetOnAxis(ap=ridx[:, s:s + 1], axis=0),
                    )

                featT = pool.tile([128, 3 * TP], bf16)
                sdst = pool.tile([SCAL, TP], bf16)
                ssrc = pool.tile([SCAL, TP], bf16)

                def fwd_rotate(dup_view, rote_ap, out_tile, eng):
                    """out[(j,k,l)] = sum_m dup[(j,k,m,l)] * rote[(k,m,l)],
                    with rote_fwd storing rot[k,l,m] at (k,m,l)."""
                    tmp = pool3.tile([SUB, ROTD], bf16, tag="tmprot")
                    r_b = (
                        rote_ap.rearrange("p (k q) -> p k q", k=L, q=4)
                        .unsqueeze(1)
                        .broadcast_to([SUB, NREP, L, 4])
                    )
                    eng.tensor_tensor(
                        out=tmp[:].rearrange("p (j k q) -> p j k q",
                                             j=NREP, k=L, q=4),
                        in0=dup_view,
                        in1=r_b,
                        op=AL.mult,
                    )
                    tv = tmp[:].rearrange("p (a m l) -> p a m l",
                                          a=NREP * L, m=2, l=2)
                    eng.tensor_tensor(
                        out=out_tile.rearrange("p (a l) -> p a l",
                                               a=NREP * L, l=2),
                        in0=tv[:, :, 0, :],
                        in1=tv[:, :, 1, :],
                        op=AL.add,
                    )

                for s in range(NSUBT):
                    cL = s * SUB
                    rfs = rote_f[:, s * 16:(s + 1) * 16]

                    # ---- dst rot features: expand + rotate ----
                    p_x1 = px.tile([SUB, ROTD], f32, tag="px")
                    nc.tensor.matmul(
                        out=p_x1[:],
                        lhsT=onehot_w[:, cL:cL + SUB],
                        rhs=win[:, SCAL:FEATD],
                        start=True,
                        stop=True,
                    )
                    dst_rot = pool3.tile([SUB, ROTF], bf16, tag="dstrot")
                    fwd_rotate(
                        p_x1[:].rearrange("p (j k q) -> p j k q",
                                          j=NREP, k=L, q=4),
                        rfs, dst_rot[:], nc.vector,
                    )

                    # ---- src rot features ----
                    sg = src_g[:, s * PACK_W:s * PACK_W + FEATD]
                    src_rot = pool3.tile([SUB, ROTF], bf16, tag="srcrot")
                    fwd_rotate(
                        sg[:, SCAL:FEATD].rearrange(
                            "p (j k q) -> p j k q", j=NREP, k=L, q=4
                        ),
                        rfs, src_rot[:], nc.vector,
                    )

                    # ---- transposes into chunk tiles ----
                    ptn = ptr.tile([128, 512], bf16, tag="ptrans")
                    nc.tensor.transpose(
                        out=ptn[:, 0:128], in_=dst_rot[:, 0:128], identity=ident[:]
                    )
                    nc.tensor.transpose(
                        out=ptn[0:64, 128:256], in_=dst_rot[:, 128:192],
                        identity=ident[:],
                    )
                    nc.tensor.transpose(
                        out=ptn[64:128, 128:256], in_=src_rot[:, 128:192],
                        identity=ident[:],
                    )
                    nc.tensor.transpose(
                        out=ptn[:, 256:384], in_=src_rot[:, 0:128], identity=ident[:]
                    )
                    nc.tensor.transpose(
                        out=ptn[0:SCAL, 384:512], in_=sg[:, 0:SCAL], identity=ident[:]
                    )
                    # merged copy of the three 128-part sections -> featT blocks
                    nc.scalar.activation(
                        out=featT[:].rearrange("p (c e) -> p c e", c=3, e=TP)[
                            :, :, cL:cL + SUB
                        ],
                        in_=ptn[:, 0:384].rearrange("p (c e) -> p c e", c=3, e=SUB),
                        func=ACTF.Copy,
                    )
                    nc.scalar.activation(
                        out=ssrc[:, cL:cL + SUB], in_=ptn[0:SCAL, 384:512],
                        func=ACTF.Copy,
                    )

                # ---- dst scalar expand (once per tile) ----
                p_x2 = pph.tile([SCAL, TP], f32, tag="ph")
                nc.tensor.matmul(
                    out=p_x2[:],
                    lhsT=win[:, 0:SCAL],
                    rhs=onehot_w[:],
                    start=True,
                    stop=True,
                )
                nc.scalar.activation(out=sdst[:], in_=p_x2[:], func=ACTF.Copy)

                # ---- MLP layer 1 + relu ----
                rhs_chunks = [
                    featT[:, 0:TP], featT[:, TP:2 * TP], featT[:, 2 * TP:3 * TP],
                    sdst[:], ssrc[:], dist_sb[:],
                ]
                hT = pool.tile([128, 2 * TP], bf16)
                for hh in range(2):
                    p_h = pph.tile([128, TP], f32, tag="ph")
                    for c in range(6):
                        nc.tensor.matmul(
                            out=p_h[:],
                            lhsT=w1sb[0:KC[c], c * HID + hh * 128:c * HID + (hh + 1) * 128],
                            rhs=rhs_chunks[c][0:KC[c], :],
                            start=(c == 0),
                            stop=(c == 5),
                        )
                    nc.scalar.activation(
                        out=hT[:, hh * TP:(hh + 1) * TP],
                        in_=p_h[:],
                        func=ACTF.Relu,
                    )

                # ---- MLP layer 2 (dup output cols, 3 partition chunks) ----
                msgT = []
                for dd, (d0, dw) in enumerate([(0, 128), (128, 128), (256, 64)]):
                    p_o = ppo.tile([dw, TP], f32, tag="po")
                    for hh in range(2):
                        nc.tensor.matmul(
                            out=p_o[:],
                            lhsT=w2sb[:, hh * DOUTD + d0:hh * DOUTD + d0 + dw],
                            rhs=hT[:, hh * TP:(hh + 1) * TP],
                            start=(hh == 0),
                            stop=(hh == 1),
                        )
                    mt = pool.tile([dw, TP], bf16, tag=f"msgT{dd}")
                    if dd == 0:
                        nc.vector.tensor_copy(out=mt[:], in_=p_o[:])
                    else:
                        nc.scalar.activation(out=mt[:], in_=p_o[:], func=ACTF.Copy)
                    msgT.append(mt)

                return dict(rote_b=rote_b, onehot_e=onehot_e,
                            winr_sc=winr_sc, msgT=msgT)

            def emit_back(st):
                rote_b = st["rote_b"]
                onehot_e = st["onehot_e"]
                winr_sc = st["winr_sc"]
                msgT = st["msgT"]
                # ---- back-rotation + scatter ----
                p_sc = psc.tile([W, DOUT], f32, tag="psc")
                for s in range(NSUBT):
                    cL = s * SUB
                    rbs = rote_b[:, s * 16:(s + 1) * 16]
                    p_m = ptr.tile([128, DOUTD], bf16, tag="ptrans")
                    nc.tensor.transpose(
                        out=p_m[:, 0:128], in_=msgT[0][:, cL:cL + SUB],
                        identity=ident[:],
                    )
                    nc.tensor.transpose(
                        out=p_m[:, 128:256], in_=msgT[1][:, cL:cL + SUB],
                        identity=ident[:],
                    )
                    nc.tensor.transpose(
                        out=p_m[:, 256:320], in_=msgT[2][:, cL:cL + SUB],
                        identity=ident[0:64, 0:64],
                    )
                    out_sb = pool3.tile([SUB, DOUT], bf16, tag="outsb")
                    nc.scalar.activation(out=out_sb[:, 0:NS], in_=p_m[:, 0:NS], func=ACTF.Copy)
                    # out[(j,k,l)] = sum_m msgdup[(j,k,m,l)] * rote_b[(k,m,l)]
                    tmpb = pool3.tile([SUB, 256], bf16, tag="tmpback")
                    r_b = (
                        rbs.rearrange("p (k q) -> p k q", k=L, q=4)
                        .unsqueeze(1)
                        .broadcast_to([SUB, NR, L, 4])
                    )
                    nc.vector.tensor_tensor(
                        out=tmpb[:].rearrange("p (j k q) -> p j k q",
                                              j=NR, k=L, q=4),
                        in0=p_m[:, NS:DOUTD].rearrange(
                            "p (j k q) -> p j k q", j=NR, k=L, q=4
                        ),
                        in1=r_b,
                        op=AL.mult,
                    )
                    tb = tmpb[:].rearrange("p (a m l) -> p a m l",
                                           a=NR * L, m=2, l=2)
                    nc.vector.tensor_tensor(
                        out=out_sb[:, NS:DOUT].rearrange(
                            "p (a l) -> p a l", a=NR * L, l=2
                        ),
                        in0=tb[:, :, 0, :],
                        in1=tb[:, :, 1, :],
                        op=AL.add,
                    )
                    nc.tensor.matmul(
                        out=p_sc[:],
                        lhsT=onehot_e[:, s * W:(s + 1) * W],
                        rhs=out_sb[:],
                        start=(s == 0),
                        stop=(s == NSUBT - 1),
                    )
                out_f = pool.tile([W, DOUT], f32)
                nc.scalar.activation(out=out_f[:], in_=p_sc[:], func=ACTF.Copy)
                nc.gpsimd.indirect_dma_start(
                    out=d_acc[:],
                    out_offset=IndirectOffsetOnAxis(ap=winr_sc, axis=0),
                    in_=out_f[:],
                    in_offset=None,
                )

            # software pipeline: emit front(t+1) before back(t) so the
            # scheduler interleaves t+1's gathers/rotations with t's MLP
            st = emit_front(0)
            for t in range(1, T):
                st_next = emit_front(t)
                emit_back(st)
                st = st_next
            emit_back(st)

    nc.compile()
    return nc


_PROGRAM_CACHE = {}


def _get_program(T):
    if T not in _PROGRAM_CACHE:
        _PROGRAM_CACHE[T] = _build_program(T)
    return _PROGRAM_CACHE[T]


class _PjrtExec:
    """Persistent jitted SPMD executable for one Bass program (axon/PJRT)."""

    def __init__(self, nc):
        import jax
        from jax.sharding import Mesh, PartitionSpec
        from jax.experimental.shard_map import shard_map
        import concourse.mybir as mybir
        from concourse.bass2jax import (
            _bass_exec_p,
            fast_dispatch_compile,
            install_neuronx_cc_hook,
            partition_id_tensor,
        )

        install_neuronx_cc_hook()
        self.nc = nc
        partition_name = (
            nc.partition_id_tensor.name if nc.partition_id_tensor else None
        )
        in_names, out_names, out_avals, zero_shapes = [], [], [], []
        for alloc in nc.m.functions[0].allocations:
            if not isinstance(alloc, mybir.MemoryLocationSet):
                continue
            name = alloc.memorylocations[0].name
            if alloc.kind == "ExternalInput":
                if name != partition_name:
                    in_names.append(name)
            elif alloc.kind == "ExternalOutput":
                shape = tuple(alloc.tensor_shape)
                dtype = mybir.dt.np(alloc.dtype)
                out_names.append(name)
                out_avals.append(jax.core.ShapedArray(shape, dtype))
                zero_shapes.append((shape, dtype))
        self.in_names = in_names
        self.out_names = out_names
        self.out_avals = out_avals
        self.zero_shapes = zero_shapes
        n_params, n_outs = len(in_names), len(out_names)
        all_names = in_names + out_names
        if partition_name is not None:
            all_names.append(partition_name)

        def _body(*args):
            operands = list(args)
            if partition_name is not None:
                operands.append(partition_id_tensor())
            outs = _bass_exec_p.bind(
                *operands,
                out_avals=tuple(out_avals),
                in_names=tuple(all_names),
                out_names=tuple(out_names),
                lowering_input_output_aliases=(),
                sim_require_finite=True,
                sim_require_nnan=True,
                nc=nc,
            )
            return tuple(outs)

        devices = jax.devices()[:NCORES]
        mesh = Mesh(np.asarray(devices), ("core",))
        self.mesh = mesh
        self.in_sharding = jax.sharding.NamedSharding(
            mesh, PartitionSpec("core")
        )
        # AOT-compile on the C++ fast-dispatch path (bass_effect suppressed)
        # so the per-call Python overhead stays small.
        in_shapes = []
        for name in in_names:
            alloc_shapes = {
                a.memorylocations[0].name: (tuple(a.tensor_shape), mybir.dt.np(a.dtype))
                for a in nc.m.functions[0].allocations
                if isinstance(a, mybir.MemoryLocationSet)
                and a.kind in ("ExternalInput", "ExternalOutput")
            }
            s, d = alloc_shapes[name]
            in_shapes.append(
                jax.ShapeDtypeStruct((NCORES * s[0], *s[1:]), d, sharding=self.in_sharding)
            )
        for (s, d) in zero_shapes:
            in_shapes.append(
                jax.ShapeDtypeStruct((NCORES * s[0], *s[1:]), d, sharding=self.in_sharding)
            )

        def _compile():
            return jax.jit(
                shard_map(
                    _body,
                    mesh=mesh,
                    in_specs=(PartitionSpec("core"),) * (n_params + n_outs),
                    out_specs=(PartitionSpec("core"),) * n_outs,
                    check_rep=False,
                ),
                keep_unused=True,
            ).lower(*in_shapes).compile()

        self.fn = fast_dispatch_compile(_compile)

    def stage_inputs(self, per_core_inputs):
        import jax

        concat_in = [
            np.concatenate(
                [np.asarray(per_core_inputs[c][n]) for c in range(NCORES)], axis=0
            )
            for n in self.in_names
        ]
        concat_in += [
            np.zeros((NCORES * s[0], *s[1:]), d) for (s, d) in self.zero_shapes
        ]
        staged = [jax.device_put(a, self.in_sharding) for a in concat_in]
        jax.block_until_ready(staged)
        return staged

    def run(self, staged):
        import jax

        outs = self.fn(*staged)
        jax.block_until_ready(outs)
        return outs

    def results(self, outs):
        res = []
        for c in range(NCORES):
            res.append(
                {
                    n: np.asarray(outs[i]).reshape(
                        NCORES, *self.out_avals[i].shape
                    )[c]
                    for i, n in enumerate(self.out_names)
                }
            )
        return res


_EXEC_CACHE = {}


def _get_exec(T):
    if T not in _EXEC_CACHE:
        _EXEC_CACHE[T] = _PjrtExec(_get_program(T))
    return _EXEC_CACHE[T]


def kernel(**inputs):
    per_core_inputs, T, meta = _host_prep(inputs)
    ex = _get_exec(T)
    staged = ex.stage_inputs(per_core_inputs)
    outs = ex.run(staged)
    return _assemble(ex.results(outs), meta)

